# revision 1
# baseline (speedup 1.0000x reference)
"""DilatedAttention Trainium2 kernel (8 NeuronCores, SPMD).

Input  : q, k, v each (2, 24, 8192, 64) float32.
Output : same shape; per head-group windowed attention over dilated
         positions, non-dilated positions zero.

Sharding: 3 head groups x (b in 2, hg in 8) = 16 (b,head) pairs per
group. Core c takes pairs {2c, 2c+1} of every group -> 6 slices per
core, perfectly balanced, no cross-core communication.

Per-core kernel: for each slice, process segments in "quads" (8
segments = 4 duos). A duo packs 2 segments on partition halves:
 - cast-DMA (f32->bf16) loads Q,K as [m, 8*64] (seg-major free dim)
   and V duo-stacked [128, 4*65] with a ones column per duo.
 - one PE transpose per duo-tensor yields Q^T/K^T for both segments
   stacked on partition halves [128, m].
 - mm1 per half: lt[k,q] = K^T.T @ Q^T   (contraction d=64)
 - exp on ACT (PSUM->SBUF bf16, scale=1/sqrt(d); no max-subtraction
   needed: logits are O(5))
 - mm2 per half: [out_un | s] = e.T @ [V | 1]  (contraction k=m)
 - reciprocal + per-partition scale on DVE, HWDGE store of the
   dilated rows only (ExternalOutput buffers are pre-zeroed).

All PSUM tiles are full-bank sized: sub-bank PSUM tiles get packed at
non-bank-aligned offsets, and a matmul output that crosses a PSUM bank
boundary is fatal on hardware.
"""

import sys

if "/opt/trn_rl_repo" not in sys.path:
    sys.path.insert(0, "/opt/trn_rl_repo")

from contextlib import ExitStack

import numpy as np

import concourse.bass as bass  # noqa: F401
import concourse.mybir as mybir
import concourse.tile as tile
from concourse import bacc
from concourse.bass_utils import run_bass_kernel_spmd
from concourse.masks import make_identity

B, H, S, D = 2, 24, 8192, 64
W_LIST = [64, 128, 256]
R_LIST = [1, 2, 4]
NG = 3
G = H // NG  # heads per group
N_CORES = 8
SCALE = 1.0 / (D**0.5)

# slice order per core: (group, pair_within_core)
SLICES = [(0, 0), (0, 1), (1, 0), (1, 1), (2, 0), (2, 1)]

F32 = mybir.dt.float32
BF16 = mybir.dt.bfloat16

_PROGRAM = None
LAST_RESULT = None  # BassKernelResults of the most recent run (for test.py)


def build_slice(nc, tc, pools, ident, qs_ap, ks_ap, vs_ap, os_ap, g, nq=None):
    """Emit the program for one (b, head) slice of group g.

    qs_ap.. are [S, D] DRAM APs for this slice.
    """
    ld, tp, sb, ps, outp = pools
    w, r = W_LIST[g], R_LIST[g]
    off = g * r
    m = len(range(off, w, r))  # 64 / 63 / 62
    mp = m + (m & 1)  # even column pitch (PSUM bf16 needs 4B alignment)
    n = qs_ap.shape[0] // w
    if nq is None:
        nq = n // 8
    # partition ranges of the two duo halves; one fused range when m == 64
    halves = [(0, 128)] if m == 64 else [(0, m), (64, 64 + m)]

    qv = qs_ap.rearrange("(n w) d -> w n d", w=w)[off::r]
    kv = ks_ap.rearrange("(n w) d -> w n d", w=w)[off::r]
    vv = vs_ap.rearrange("(n w) d -> w n d", w=w)[off::r]
    ov = os_ap.rearrange("(n w) d -> w n d", w=w)[off::r]

    for t in range(nq):
        s0 = 8 * t
        # ---- loads (f32; Q/K on the two HWDGE rings, V on SWDGE which
        # merges partition-spanning descriptors; bf16 via bitcast views) ----
        qd = ld.tile([m, 512], F32, tag="qd")
        nc.sync.dma_start(out=qd[:], in_=qv[:, s0 : s0 + 8, :])
        kd = ld.tile([m, 512], F32, tag="kd")
        nc.scalar.dma_start(out=kd[:], in_=kv[:, s0 : s0 + 8, :])
        vd = ld.tile([128, 260], F32, tag="vd")
        vdv = vd[:].rearrange("p (u e) -> p u e", e=65)
        nc.gpsimd.dma_start(out=vdv[0:m, :, 0:64], in_=vv[:, s0 : s0 + 8 : 2, :])
        nc.gpsimd.dma_start(
            out=vdv[64 : 64 + m, :, 0:64], in_=vv[:, s0 + 1 : s0 + 8 : 2, :]
        )
        nc.vector.memset(vd[:, 64:260:65], 1.0)
        qb = qd[:].bitcast(BF16)[:, 1::2]  # [m, 512] bf16 (truncated)
        kb = kd[:].bitcast(BF16)[:, 1::2]
        vb = vd[:].bitcast(BF16)[:, 1::2]  # [128, 260] bf16

        # ---- transposes: Q^T/K^T duo-stacked [128, m] each ----
        qkt_ps = tp.tile([128, 1024], BF16, tag="qkt")  # full 2KB bank
        for j in range(4):
            nc.tensor.transpose(
                qkt_ps[:, j * mp : j * mp + m],
                qb[:, j * 128 : (j + 1) * 128],
                ident[0:m, 0:m],
            )
            nc.tensor.transpose(
                qkt_ps[:, (4 + j) * mp : (4 + j) * mp + m],
                kb[:, j * 128 : (j + 1) * 128],
                ident[0:m, 0:m],
            )
        qkt = sb.tile([128, 8 * mp], BF16, tag="qkt_s")
        if mp == m:
            nc.vector.tensor_copy(qkt[:], qkt_ps[:, 0 : 8 * mp])
        else:  # strided copy skips the uninitialized pad column per block
            nc.vector.tensor_copy(
                qkt[:].rearrange("p (u x) -> p u x", x=mp)[:, :, 0:m],
                qkt_ps[:, 0 : 8 * mp].rearrange("p (u x) -> p u x", x=mp)[
                    :, :, 0:m
                ],
            )

        # ---- mm1: lt[k, q] per duo-half ----
        lt = ps.tile([128, 512], F32, tag="lt")  # full bank
        for j in range(4):
            qss = qkt[:, j * mp : j * mp + m]
            kss = qkt[:, (4 + j) * mp : (4 + j) * mp + m]
            nc.tensor.matmul(
                lt[0:m, j * m : (j + 1) * m],
                kss[0:64, :],
                qss[0:64, :],
                start=True,
                stop=True,
                tile_position=(0, 0),
            )
            nc.tensor.matmul(
                lt[64 : 64 + m, j * m : (j + 1) * m],
                kss[64:128, :],
                qss[64:128, :],
                start=True,
                stop=True,
                tile_position=(64, 64),
            )

        # ---- softmax numerator (per half: avoid unwritten partitions) ----
        e = sb.tile([128, 4 * mp], BF16, tag="e")
        for p0, p1 in halves:
            if mp == m:
                nc.scalar.activation(
                    e[p0:p1, :],
                    lt[p0:p1, 0 : 4 * m],
                    mybir.ActivationFunctionType.Exp,
                    scale=SCALE,
                )
            else:
                ev = e[p0:p1, :].rearrange("p (u x) -> p u x", x=mp)[:, :, 0:m]
                lv = lt[p0:p1, 0 : 4 * m].rearrange("p (u x) -> p u x", x=m)
                nc.scalar.activation(
                    ev, lv, mybir.ActivationFunctionType.Exp, scale=SCALE
                )

        # ---- mm2: [out_un | s] = e.T @ [V | 1] per duo-half ----
        o_ps = ps.tile([128, 512], F32, tag="ops")  # full bank
        for j in range(4):
            nc.tensor.matmul(
                o_ps[0:m, j * 65 : (j + 1) * 65],
                e[0:m, j * mp : j * mp + m],
                vb[0:m, j * 65 : (j + 1) * 65],
                start=True,
                stop=True,
                tile_position=(0, 0),
            )
            nc.tensor.matmul(
                o_ps[64 : 64 + m, j * 65 : (j + 1) * 65],
                e[64 : 64 + m, j * mp : j * mp + m],
                vb[64 : 64 + m, j * 65 : (j + 1) * 65],
                start=True,
                stop=True,
                tile_position=(64, 64),
            )

        # ---- normalize + store ----
        rcp = sb.tile([128, 4], F32, tag="rcp")
        ost = outp.tile([128, 256], F32, tag="ost")
        for p0, p1 in halves:
            nc.vector.reciprocal(rcp[p0:p1, :], o_ps[p0:p1, 64:260:65])
            for j in range(4):
                nc.vector.tensor_scalar_mul(
                    ost[p0:p1, j * 64 : (j + 1) * 64],
                    o_ps[p0:p1, j * 65 : j * 65 + 64],
                    rcp[p0:p1, j : j + 1],
                )
        nc.sync.dma_start(out=ov[:, s0 : s0 + 8 : 2, :], in_=ost[0:m, :])
        nc.scalar.dma_start(
            out=ov[:, s0 + 1 : s0 + 8 : 2, :], in_=ost[64 : 64 + m, :]
        )


def make_pools(tc, stack):
    ld = stack.enter_context(tc.tile_pool(name="ld", bufs=12))
    tp = stack.enter_context(tc.tile_pool(name="tp", bufs=2, space="PSUM"))
    sb = stack.enter_context(tc.tile_pool(name="sb", bufs=4))
    ps = stack.enter_context(tc.tile_pool(name="ps", bufs=2, space="PSUM"))
    outp = stack.enter_context(tc.tile_pool(name="outp", bufs=4))
    return ld, tp, sb, ps, outp


def _build_program():
    nc = bacc.Bacc("TRN2", target_bir_lowering=False, debug=False)
    q = nc.dram_tensor("q", [6, S, D], F32, kind="ExternalInput").ap()
    k = nc.dram_tensor("k", [6, S, D], F32, kind="ExternalInput").ap()
    v = nc.dram_tensor("v", [6, S, D], F32, kind="ExternalInput").ap()
    o = nc.dram_tensor("o", [6, S, D], F32, kind="ExternalOutput").ap()

    with tile.TileContext(nc) as tc:
        with ExitStack() as stack:
            cpool = stack.enter_context(tc.tile_pool(name="const", bufs=1))
            ident = cpool.tile([64, 64], BF16)
            make_identity(nc, ident[:])
            pools = make_pools(tc, stack)
            for sl, (g, _pair) in enumerate(SLICES):
                build_slice(nc, tc, pools, ident, q[sl], k[sl], v[sl], o[sl], g)

    nc.finalize()
    return nc


def _get_program():
    global _PROGRAM
    if _PROGRAM is None:
        _PROGRAM = _build_program()
    return _PROGRAM


def kernel(q, k, v):
    global LAST_RESULT
    q = np.asarray(q, dtype=np.float32)
    k = np.asarray(k, dtype=np.float32)
    v = np.asarray(v, dtype=np.float32)
    assert q.shape == (B, H, S, D), q.shape

    nc = _get_program()

    # (b, head) pair p = b*G + hg within group g; core c owns p in {2c, 2c+1}
    in_maps = []
    for c in range(N_CORES):
        qc = np.empty((6, S, D), np.float32)
        kc = np.empty((6, S, D), np.float32)
        vc = np.empty((6, S, D), np.float32)
        for sl, (g, j) in enumerate(SLICES):
            p = 2 * c + j
            b, hg = p // G, p % G
            head = g * G + hg
            qc[sl] = q[b, head]
            kc[sl] = k[b, head]
            vc[sl] = v[b, head]
        in_maps.append({"q": qc, "k": kc, "v": vc})

    LAST_RESULT = run_bass_kernel_spmd(nc, in_maps, core_ids=list(range(N_CORES)))

    out = np.zeros((B, H, S, D), np.float32)
    for c in range(N_CORES):
        oc = LAST_RESULT.results[c]["o"]
        for sl, (g, j) in enumerate(SLICES):
            p = 2 * c + j
            b, hg = p // G, p % G
            head = g * G + hg
            out[b, head] = oc[sl]
    return out

